# revision 6
# baseline (speedup 1.0000x reference)
"""Single-head causal attention (S=8192, D=E=1024, fp32) on 8 trn2 NeuronCores.

Returns (z, (scores, Q, V, K)) matching the reference pytree.

Sharding: core c owns row-blocks {8k+c : k in 0..7} (128 rows each). Block k
computes score columns [0, 1024*(k+1)) — identical loop bounds on every core
(uniform SPMD program), causal work perfectly balanced across cores. The
causal boundary inside the diagonal 1024-col group depends only on c, so it
is applied via a host-supplied [128,1024] additive bias input.

Per core: project K/V for a contiguous 1024-row chunk, Q^T for its own rows;
AllGather K^T and V chunks across cores; stream K^T/V column groups from the
gathered copy; QK on PE, exp+rowsum fused on ACT, normalize on DVE, z via
PE-transposed score tiles.
"""

import sys

if "/opt/trn_rl_repo" not in sys.path:
    sys.path.insert(0, "/opt/trn_rl_repo")

import numpy as np

S, D, E = 8192, 1024, 1024
P = 128
NCORES = 8
RB = 8  # row blocks per core
NG = 8  # column groups of 1024
HALVES = [(range(0, 5), 5), (range(5, 8), 8)]  # (blocks, n_groups)
NEG = -3.0e38

_CACHE = {}


def _split_excess_waits(nc, max_waits=1):
    """The pinned walrus rejects >1 sync-wait on some opcodes; hoist extras
    onto preceding NOPs on the same engine."""
    import bass_rust
    import concourse.mybir as mybir

    for fn in nc.m.functions:
        for bb in fn.blocks:
            insts = list(bb.instructions)
            out, changed = [], False
            for ins in insts:
                si = ins.sync_info
                if si is not None and len(si.on_wait) > max_waits:
                    waits = list(si.on_wait)
                    extra, keep = waits[:-max_waits], waits[-max_waits:]
                    for j, w in enumerate(extra):
                        nop = mybir.InstNoOp(
                            name=f"{ins.name}-wsplit{j}", ins=[], outs=[]
                        )
                        nop.engine = ins.engine
                        nop.sync_info = bass_rust.SyncInfo(on_wait=[w], on_update=[])
                        out.append(nop)
                    si.on_wait = keep
                    changed = True
                out.append(ins)
            if changed:
                bb.instructions = out


def build():
    import concourse.bass as bass
    import concourse.mybir as mybir
    import concourse.tile as tile
    from concourse.masks import make_identity

    f32 = mybir.dt.float32
    ADD = mybir.AluOpType.add
    X = mybir.AxisListType.X
    EXP = mybir.ActivationFunctionType.Exp

    nc = bass.Bass(target_bir_lowering=False)

    x_q = nc.dram_tensor("x_q", [1024, D], f32, kind="ExternalInput")
    x_kv = nc.dram_tensor("x_kv", [1024, D], f32, kind="ExternalInput")
    wq_d = nc.dram_tensor("wq", [D, E], f32, kind="ExternalInput")
    wk_d = nc.dram_tensor("wk", [D, E], f32, kind="ExternalInput")
    wv_d = nc.dram_tensor("wv", [D, E], f32, kind="ExternalInput")
    maskb = nc.dram_tensor("maskb", [P, 1024], f32, kind="ExternalInput")

    q_out = nc.dram_tensor("q_out", [1024, E], f32, kind="ExternalOutput")
    k_out = nc.dram_tensor("k_out", [1024, E], f32, kind="ExternalOutput")
    v_out = nc.dram_tensor("v_out", [1024, E], f32, kind="ExternalOutput")
    z_out = nc.dram_tensor("z_out", [1024, E], f32, kind="ExternalOutput")
    s_out = nc.dram_tensor("s_out", [1024, S], f32, kind="ExternalOutput")

    with tile.TileContext(nc) as tc:
        with (
            tc.tile_pool(name="persist", bufs=1) as persist,
            tc.tile_pool(name="dram", bufs=1, space="DRAM") as dram,
        ):
            ident = persist.tile([P, P], f32)
            make_identity(nc, ident[:])
            mask_sb = persist.tile([P, 1024], f32)
            nc.sync.dma_start(mask_sb[:], maskb[:])
            qt_sb = persist.tile([P, 8, 1024], f32)  # [e_lo, e_hi, row]

            kt_bounce = dram.tile([E, 1024], f32, name="kt_bounce")
            kt_ag = dram.tile([NCORES, E, 1024], f32, addr_space="Shared",
                              name="kt_ag")
            v_bounce = dram.tile([1024, E], f32, name="v_bounce")
            v_ag = dram.tile([NCORES, 1024, E], f32, addr_space="Shared",
                             name="v_ag")

            # ---------------- stage A: projections -----------------
            with (
                tc.tile_pool(name="stageA", bufs=4) as big,
                tc.tile_pool(name="natp", bufs=3) as natp,
                tc.tile_pool(name="psT_A", bufs=4, space="PSUM") as psT_A,
                tc.tile_pool(name="psMM_A", bufs=3, space="PSUM") as psMM_A,
            ):
                def transpose_128(src_ap, dst_ap, tag="ta"):
                    pt = psT_A.tile([P, P], f32, tag=tag, name=f"pt_{tag}")
                    nc.tensor.transpose(pt[:], src_ap, ident[:])
                    nc.any.tensor_copy(dst_ap, pt[:])

                # x_kv -> xT_kv
                xkv_sb = big.tile([P, 8, 1024], f32, tag="big", name="xkv")
                nc.sync.dma_start(
                    xkv_sb[:], x_kv.rearrange("(rh rl) d -> rl rh d", rl=P))
                xt_kv = big.tile([P, 8, 1024], f32, tag="big", name="xtkv")
                for rh in range(8):
                    for dh in range(8):
                        transpose_128(xkv_sb[:, rh, dh * P:(dh + 1) * P],
                                      xt_kv[:, dh, rh * P:(rh + 1) * P])

                # KT = wk^T @ x_kv^T  -> [e, s]
                wk_sb = big.tile([P, 8, 1024], f32, tag="big", name="wk_sb")
                nc.sync.dma_start(
                    wk_sb[:], wk_d.rearrange("(dh dl) e -> dl dh e", dl=P))
                kt_sb = big.tile([P, 8, 1024], f32, tag="big", name="kt_sb")
                for eh in range(8):
                    for s2 in range(2):
                        pm = psMM_A.tile([P, 512], f32, tag="pm", name="pm_kt")
                        for dh in range(8):
                            nc.tensor.matmul(
                                pm[:],
                                lhsT=wk_sb[:, dh, eh * P:(eh + 1) * P],
                                rhs=xt_kv[:, dh, s2 * 512:(s2 + 1) * 512],
                                start=(dh == 0), stop=(dh == 7))
                        nc.any.tensor_copy(
                            kt_sb[:, eh, s2 * 512:(s2 + 1) * 512], pm[:])
                # bounce KT (as [E, s] row-major) and AllGather early
                nc.sync.dma_start(
                    kt_bounce.rearrange("(eh el) s -> el eh s", el=P), kt_sb[:])
                nc.gpsimd.collective_compute(
                    "AllGather", mybir.AluOpType.bypass,
                    replica_groups=[list(range(NCORES))],
                    ins=[kt_bounce.opt()], outs=[kt_ag.opt()])

                # K natural output: transpose KT tiles via PSUM + SBUF stage
                for st in range(8):
                    knat = natp.tile([P, 1024], f32, tag="nat", name="knat")
                    for eh in range(8):
                        transpose_128(kt_sb[:, eh, st * P:(st + 1) * P],
                                      knat[:, eh * P:(eh + 1) * P])
                    nc.sync.dma_start(k_out[st * P:(st + 1) * P, :], knat[:])

                # V = x_kv @ wv -> [s, e], straight to v_out + bounce
                wv_sb = big.tile([P, 8, 1024], f32, tag="big", name="wv_sb")
                nc.sync.dma_start(
                    wv_sb[:], wv_d.rearrange("(dh dl) e -> dl dh e", dl=P))
                for rt in range(8):
                    vnat = natp.tile([P, 1024], f32, tag="nat", name="vnat")
                    for e2 in range(2):
                        pm = psMM_A.tile([P, 512], f32, tag="pm", name="pm_v")
                        for dh in range(8):
                            nc.tensor.matmul(
                                pm[:],
                                lhsT=xt_kv[:, dh, rt * P:(rt + 1) * P],
                                rhs=wv_sb[:, dh, e2 * 512:(e2 + 1) * 512],
                                start=(dh == 0), stop=(dh == 7))
                        nc.any.tensor_copy(
                            vnat[:, e2 * 512:(e2 + 1) * 512], pm[:])
                    nc.sync.dma_start(v_out[rt * P:(rt + 1) * P, :], vnat[:])
                    nc.sync.dma_start(
                        v_bounce[rt * P:(rt + 1) * P, :], vnat[:])
                nc.gpsimd.collective_compute(
                    "AllGather", mybir.AluOpType.bypass,
                    replica_groups=[list(range(NCORES))],
                    ins=[v_bounce.opt()], outs=[v_ag.opt()])

                # x_q -> xT_q ; QT = wq^T @ x_q^T (persistent)
                xq_sb = big.tile([P, 8, 1024], f32, tag="big", name="xq")
                nc.sync.dma_start(
                    xq_sb[:], x_q.rearrange("(rh rl) d -> rl rh d", rl=P))
                xt_q = big.tile([P, 8, 1024], f32, tag="big", name="xtq")
                for rh in range(8):
                    for dh in range(8):
                        transpose_128(xq_sb[:, rh, dh * P:(dh + 1) * P],
                                      xt_q[:, dh, rh * P:(rh + 1) * P])
                wq_sb = big.tile([P, 8, 1024], f32, tag="big", name="wq_sb")
                nc.sync.dma_start(
                    wq_sb[:], wq_d.rearrange("(dh dl) e -> dl dh e", dl=P))
                for eh in range(8):
                    for r2 in range(2):
                        pm = psMM_A.tile([P, 512], f32, tag="pm", name="pm_q")
                        for dh in range(8):
                            nc.tensor.matmul(
                                pm[:],
                                lhsT=wq_sb[:, dh, eh * P:(eh + 1) * P],
                                rhs=xt_q[:, dh, r2 * 512:(r2 + 1) * 512],
                                start=(dh == 0), stop=(dh == 7))
                        nc.any.tensor_copy(
                            qt_sb[:, eh, r2 * 512:(r2 + 1) * 512], pm[:])
                # Q natural output from QT
                for rt in range(8):
                    qnat = natp.tile([P, 1024], f32, tag="nat", name="qnat")
                    for eh in range(8):
                        transpose_128(qt_sb[:, eh, rt * P:(rt + 1) * P],
                                      qnat[:, eh * P:(eh + 1) * P])
                    nc.sync.dma_start(q_out[rt * P:(rt + 1) * P, :], qnat[:])

            # ---------------- attention -----------------
            with (
                tc.tile_pool(name="stream", bufs=3) as stream,
                tc.tile_pool(name="stp", bufs=2) as st_pool,
                tc.tile_pool(name="stats", bufs=8) as stats,
                tc.tile_pool(name="psQK", bufs=3, space="PSUM") as psQK,
                tc.tile_pool(name="psTr", bufs=2, space="PSUM") as psTr,
                tc.tile_pool(name="psZ", bufs=2, space="PSUM") as psZ,
            ):
                for blocks, ngroups in HALVES:
                    with (
                        tc.tile_pool(name="scores", bufs=1) as sc_pool,
                        tc.tile_pool(name="zpool", bufs=1) as z_pool,
                    ):
                        sc = {k: sc_pool.tile([P, (k + 1) * 1024], f32,
                                              name=f"sc{k}")
                              for k in blocks}
                        z_sb = {k: z_pool.tile([P, 1024], f32, name=f"z{k}")
                                for k in blocks}
                        # phase 1: QK + mask into scores
                        for g in range(ngroups):
                            for sh in range(2):
                                ktt = stream.tile([P, 8, 512], f32,
                                                  tag="stream",
                                                  name=f"kt{g}_{sh}")
                                nc.sync.dma_start(
                                    ktt[:],
                                    kt_ag[g].rearrange(
                                        "(eh el) s -> el eh s", el=P
                                    )[:, :, sh * 512:(sh + 1) * 512])
                                for k in blocks:
                                    if k < g:
                                        continue
                                    pm = psQK.tile([P, 512], f32, tag="qk",
                                                   name="pm_qk")
                                    for eh in range(8):
                                        nc.tensor.matmul(
                                            pm[:],
                                            lhsT=qt_sb[:, eh,
                                                       k * P:(k + 1) * P],
                                            rhs=ktt[:, eh, :],
                                            start=(eh == 0), stop=(eh == 7))
                                    dst = sc[k][:, g * 1024 + sh * 512:
                                                g * 1024 + (sh + 1) * 512]
                                    if g == k:
                                        nc.vector.tensor_tensor(
                                            dst, pm[:],
                                            mask_sb[:, sh * 512:
                                                    (sh + 1) * 512],
                                            ADD)
                                    else:
                                        nc.any.tensor_copy(dst, pm[:])
                            # block g complete -> softmax + scores out
                            if g in blocks:
                                k = g
                                W = (k + 1) * 1024
                                negmax = stats.tile([P, 1], f32, tag="stat",
                                                    name="negmax")
                                nc.vector.reduce_max(negmax[:], sc[k][:, :W],
                                                     axis=X, negate=True)
                                bias = stats.tile([P, 1], f32, tag="stat",
                                                  name="bias")
                                nc.vector.tensor_scalar_mul(
                                    bias[:], negmax[:], 1.0 / 32.0)
                                ssum = stats.tile([P, 1], f32, tag="stat",
                                                  name="ssum")
                                nc.scalar.activation(
                                    sc[k][:, :W], sc[k][:, :W], EXP,
                                    bias=bias[:], scale=1.0 / 32.0,
                                    accum_out=ssum[:])
                                rinv = stats.tile([P, 1], f32, tag="stat",
                                                  name="rinv")
                                nc.vector.reciprocal(rinv[:], ssum[:])
                                nc.vector.tensor_scalar_mul(
                                    sc[k][:, :W], sc[k][:, :W], rinv[:])
                                nc.sync.dma_start(
                                    s_out[k * P:(k + 1) * P, :W],
                                    sc[k][:, :W])
                        # phase 3: z = scores @ V
                        for g in range(ngroups):
                            vt = []
                            for h in range(2):
                                v_t = stream.tile([P, 8, 512], f32,
                                                  tag="stream",
                                                  name=f"v{g}_{h}")
                                nc.sync.dma_start(
                                    v_t[:],
                                    v_ag[g].rearrange(
                                        "(sh sl) e -> sl sh e", sl=P
                                    )[:, :, h * 512:(h + 1) * 512])
                                vt.append(v_t)
                            for k in blocks:
                                if k < g:
                                    continue
                                st = st_pool.tile([P, 8, P], f32, tag="st",
                                                  name="st")
                                for ct in range(8):
                                    pt = psTr.tile([P, P], f32, tag="tr",
                                                   name="pt_s")
                                    nc.tensor.transpose(
                                        pt[:],
                                        sc[k][:, g * 1024 + ct * P:
                                              g * 1024 + (ct + 1) * P],
                                        ident[:])
                                    nc.any.tensor_copy(st[:, ct, :], pt[:])
                                for h in range(2):
                                    pz = psZ.tile([P, 512], f32, tag="pz",
                                                  name="pm_z")
                                    for ct in range(8):
                                        nc.tensor.matmul(
                                            pz[:],
                                            lhsT=st[:, ct, :],
                                            rhs=vt[h][:, ct, :],
                                            start=(ct == 0), stop=(ct == 7))
                                    zdst = z_sb[k][:, h * 512:(h + 1) * 512]
                                    if g == 0:
                                        nc.vector.tensor_copy(zdst, pz[:])
                                    else:
                                        nc.vector.tensor_tensor(
                                            zdst, zdst, pz[:], ADD)
                            if g in blocks:
                                nc.sync.dma_start(
                                    z_out[g * P:(g + 1) * P, :],
                                    z_sb[g][:])

    _split_excess_waits(nc)
    return nc


def make_in_maps(x, wq, wk, wv):
    x = np.ascontiguousarray(np.asarray(x, dtype=np.float32))
    wq = np.ascontiguousarray(np.asarray(wq, dtype=np.float32))
    wk = np.ascontiguousarray(np.asarray(wk, dtype=np.float32))
    wv = np.ascontiguousarray(np.asarray(wv, dtype=np.float32))
    xv = x.reshape(8, NCORES, P, D)
    cols = np.arange(1024)[None, :]
    rows = np.arange(P)[:, None]
    in_maps = []
    for c in range(NCORES):
        mask = np.where(cols <= rows + P * c, 0.0, NEG).astype(np.float32)
        in_maps.append({
            "x_q": np.ascontiguousarray(xv[:, c].reshape(1024, D)),
            "x_kv": x[1024 * c:1024 * (c + 1)],
            "wq": wq, "wk": wk, "wv": wv,
            "maskb": mask,
        })
    return in_maps


def assemble(results):
    Q = np.empty((S, E), np.float32)
    Z = np.empty((S, E), np.float32)
    K = np.empty((S, E), np.float32)
    V = np.empty((S, E), np.float32)
    SC = np.empty((S, S), np.float32)
    Qv = Q.reshape(8, NCORES, P, E)
    Zv = Z.reshape(8, NCORES, P, E)
    SCv = SC.reshape(8, NCORES, P, S)
    for c, r in enumerate(results):
        Qv[:, c] = r["q_out"].reshape(8, P, E)
        Zv[:, c] = r["z_out"].reshape(8, P, E)
        SCv[:, c] = r["s_out"].reshape(8, P, S)
        K[1024 * c:1024 * (c + 1)] = r["k_out"]
        V[1024 * c:1024 * (c + 1)] = r["v_out"]
    return (Z, (SC, Q, V, K))


def kernel(x, wq, wk, wv):
    from concourse.bass_utils import run_bass_kernel_spmd

    if "nc" not in _CACHE:
        _CACHE["nc"] = build()
    nc = _CACHE["nc"]
    in_maps = make_in_maps(x, wq, wk, wv)
    res = run_bass_kernel_spmd(nc, in_maps, core_ids=list(range(NCORES)))
    return assemble(res.results)
